# revision 11
# baseline (speedup 1.0000x reference)
"""CenterLoss on 8 Trainium2 NeuronCores (Bass/Tile).

loss = clip(distmat * onehot(labels), 1e-12, 1e12).sum() / B
     = (sum_i clip(||x_i - c_{y_i}||^2, 1e-12, 1e12) + B*(C-1)*1e-12) / B

Data-parallel over the batch: each of the 8 cores gets 4096 rows of x and
labels plus the replicated centers table.  The label-selected center rows
are fetched with the dma_gather GpSimd ucode in 4 chunks of 1024 rows —
measured at ~1.08us/128 rows sustained vs ~1.40us/128 rows for per-tile
indirect DMAs (chunking amortizes the SWDGE fixed cost and ring-reclaim
gaps).  A 16-row dummy gather at program start forces the GpSimd library
load while the x/label DMAs stream, so the real gathers start hot.  Per
128-row tile the vector engine computes x-c and the scalar engine
squares with a fused per-sample row-sum; distances are clipped on-device
and the 8 per-core partial scalars are summed on the host (the
sanctioned all-reduce).

Profiling notes (trn2): SWDGE descriptor generation is the critical path
at ~8.4-10.3ns/row serial on the GpSimd engine; a single 4096-row
dma_gather crashes the ucode (keep chunks <= 1024); multi-column offset
APs on indirect_dma_start corrupt data; an exact onehot-matmul gather on
the TensorEngine is ~3x slower (LDWEIGHTS exposed behind same-bank
accumulating matmuls).
"""

import numpy as np

BATCH, NUM_CLASSES, FEATURE_DIM = 32768, 1024, 256
N_CORES = 8
SHARD = BATCH // N_CORES  # 4096
P = 128
N_TILES = SHARD // P  # 32
N_CHUNKS = 4
CHUNK = SHARD // N_CHUNKS  # 1024 rows per dma_gather
TPC = CHUNK // P  # tiles per chunk = 8
ICOLS = CHUNK // 16  # idx columns per chunk
CLAMP_MIN, CLAMP_MAX = 1e-12, 1e12

_CACHE: dict = {}


def _build_nc():
    import concourse.bacc as bacc
    import concourse.tile as tile
    from concourse import mybir

    f32 = mybir.dt.float32
    i16 = mybir.dt.int16

    nc = bacc.Bacc("TRN2", target_bir_lowering=False, debug=False)

    x_d = nc.dram_tensor("x", [SHARD, FEATURE_DIM], f32, kind="ExternalInput")
    # dma_gather idx table: idx[16c + q, s] = labels[chunk*CHUNK + s*16 + q]
    # replicated for the 8 GpSimd cores (c in 0..7), chunks side by side
    idx_d = nc.dram_tensor("labidx", [P, SHARD // 16], i16, kind="ExternalInput")
    cen_d = nc.dram_tensor(
        "centers", [NUM_CLASSES, FEATURE_DIM], f32, kind="ExternalInput"
    )
    out_d = nc.dram_tensor("out", [1, 1], f32, kind="ExternalOutput")

    with tile.TileContext(nc) as tc:
        with (
            tc.tile_pool(name="data", bufs=N_CHUNKS) as data,
            tc.tile_pool(name="gbuf", bufs=3) as gbuf,
            tc.tile_pool(name="work", bufs=8) as work,
            tc.tile_pool(name="single", bufs=1) as single,
            tc.tile_pool(name="psum", bufs=1, space="PSUM") as psum,
        ):
            idx_all = single.tile([P, SHARD // 16], i16)
            nc.sync.dma_start(out=idx_all[:], in_=idx_d[:, :])

            # dummy 16-row gather: forces the GpSimd ucode library load at
            # t~0 so the real gathers don't stall on it mid-kernel
            scrap = single.tile([P, 1, FEATURE_DIM], f32)
            nc.gpsimd.dma_gather(
                out_ap=scrap[:],
                in_ap=cen_d[:, :],
                idxs_ap=idx_all[:, 0:1],
                num_idxs=16,
                num_idxs_reg=16,
                elem_size=FEATURE_DIM,
            )

            x_tiles = []
            for c in range(N_CHUNKS):
                x_t = data.tile([P, TPC, FEATURE_DIM], f32, tag="x")
                nc.sync.dma_start(
                    out=x_t[:],
                    in_=x_d[c * CHUNK : (c + 1) * CHUNK, :].rearrange(
                        "(t p) e -> p t e", p=P
                    ),
                )
                x_tiles.append(x_t)

            acc = single.tile([P, N_TILES], f32)
            for c in range(N_CHUNKS):
                g_t = gbuf.tile([P, TPC, FEATURE_DIM], f32, tag="g")
                nc.gpsimd.dma_gather(
                    out_ap=g_t[:],
                    in_ap=cen_d[:, :],
                    idxs_ap=idx_all[:, c * ICOLS : (c + 1) * ICOLS],
                    num_idxs=CHUNK,
                    num_idxs_reg=CHUNK,
                    elem_size=FEATURE_DIM,
                )
                for j in range(TPC):
                    t = c * TPC + j
                    d_t = work.tile([P, FEATURE_DIM], f32, tag="d")
                    nc.vector.tensor_tensor(
                        out=d_t[:],
                        in0=x_tiles[c][:, j, :],
                        in1=g_t[:, j, :],
                        op=mybir.AluOpType.subtract,
                    )
                    s_t = work.tile([P, FEATURE_DIM], f32, tag="s")
                    nc.scalar.activation(
                        out=s_t[:],
                        in_=d_t[:],
                        func=mybir.ActivationFunctionType.Square,
                        accum_out=acc[:, t : t + 1],
                    )

            clipped = single.tile([P, N_TILES], f32)
            nc.vector.tensor_scalar(
                out=clipped[:],
                in0=acc[:],
                scalar1=float(CLAMP_MIN),
                scalar2=float(CLAMP_MAX),
                op0=mybir.AluOpType.max,
                op1=mybir.AluOpType.min,
            )
            rowsum = single.tile([P, 1], f32)
            nc.vector.reduce_sum(out=rowsum[:], in_=clipped[:], axis=mybir.AxisListType.X)

            ones = single.tile([P, 1], f32)
            nc.vector.memset(ones[:], 1.0)
            tot = psum.tile([1, 1], f32, space="PSUM")
            nc.tensor.matmul(out=tot[:], lhsT=rowsum[:], rhs=ones[:], start=True, stop=True)
            res = single.tile([1, 1], f32)
            nc.vector.tensor_copy(out=res[:], in_=tot[:])
            nc.sync.dma_start(out=out_d[:, :], in_=res[:])

    nc.finalize()
    return nc


def _make_idx_table(labels_shard: np.ndarray) -> np.ndarray:
    """[SHARD] int -> [128, SHARD//16] int16 dma_gather index table."""
    tab = np.empty((P, SHARD // 16), dtype=np.int16)
    for c in range(N_CHUNKS):
        chunk = labels_shard[c * CHUNK : (c + 1) * CHUNK].astype(np.int16)
        block = chunk.reshape(ICOLS, 16).T  # [16, ICOLS]
        tab[:, c * ICOLS : (c + 1) * ICOLS] = np.tile(block, (P // 16, 1))
    return np.ascontiguousarray(tab)


def kernel(x: np.ndarray, centers: np.ndarray, labels: np.ndarray) -> np.ndarray:
    from concourse import bass_utils

    if "nc" not in _CACHE:
        _CACHE["nc"] = _build_nc()
    nc = _CACHE["nc"]

    x = np.ascontiguousarray(np.asarray(x, dtype=np.float32))
    centers = np.ascontiguousarray(np.asarray(centers, dtype=np.float32))
    lab = np.asarray(labels).astype(np.int64).reshape(N_CORES, SHARD)

    xs = x.reshape(N_CORES, SHARD, FEATURE_DIM)
    in_maps = [
        {
            "x": np.ascontiguousarray(xs[c]),
            "labidx": _make_idx_table(lab[c]),
            "centers": centers,
        }
        for c in range(N_CORES)
    ]

    rr = bass_utils.run_bass_kernel_spmd(nc, in_maps, list(range(N_CORES)))
    _CACHE["last_results"] = rr

    total = sum(float(r["out"][0, 0]) for r in rr.results)
    loss = (total + BATCH * (NUM_CLASSES - 1) * CLAMP_MIN) / BATCH
    return np.asarray(loss, dtype=np.float32)


# revision 12
# speedup vs baseline: 1.1621x; 1.1621x over previous
"""CenterLoss on 8 Trainium2 NeuronCores (Bass/Tile).

loss = clip(distmat * onehot(labels), 1e-12, 1e12).sum() / B
     = (sum_i clip(||x_i - c_{y_i}||^2, 1e-12, 1e12) + B*(C-1)*1e-12) / B

Data-parallel over the batch: each of the 8 cores gets 4096 rows of x and
labels plus the replicated centers table.  x streams in via 4 big DMAs;
the label-selected center rows are fetched 128 at a time with indirect
DMAs — the GpSimd SWDGE descriptor generation (~1.1us per 128 rows plus
~0.3us ring-reclaim gap) is the critical path, and every other engine's
work hides underneath it: per 128-row tile the vector engine computes
x-c and the scalar engine squares with a fused per-sample row-sum.
Per-sample distances are clipped on-device; the 8 per-core partial
scalars are summed on the host (the sanctioned scalar all-reduce).

Profiling notes (trn2, measured): SWDGE descriptor generation is serial
on the GpSimd engine at ~8.4-10.3ns/row for every gather variant; a
single 4096-row dma_gather crashes the ucode; chunked 1024-row
dma_gather gathers sustain ~1.08us/128 rows but stall ~20us before the
first chunk and pay a ~20us GpSimd library load; multi-column offset APs
on indirect_dma_start corrupt data (descriptor/dest zip mismatch); an
exact onehot-matmul gather on the TensorEngine runs ~3x slower than
SWDGE (LDWEIGHTS exposed behind same-bank accumulating matmuls, HAM
cold-clock).  Hence per-tile indirect DMAs with deep buffering.
"""

import numpy as np

BATCH, NUM_CLASSES, FEATURE_DIM = 32768, 1024, 256
N_CORES = 8
SHARD = BATCH // N_CORES  # 4096
P = 128
N_TILES = SHARD // P  # 32
GROUP = 8  # tiles per x-DMA
N_GROUPS = N_TILES // GROUP
CLAMP_MIN, CLAMP_MAX = 1e-12, 1e12

_CACHE: dict = {}


def _build_nc():
    import concourse.bacc as bacc
    import concourse.bass as bass
    import concourse.tile as tile
    from concourse import mybir

    f32 = mybir.dt.float32
    i32 = mybir.dt.int32

    nc = bacc.Bacc("TRN2", target_bir_lowering=False, debug=False)

    x_d = nc.dram_tensor("x", [SHARD, FEATURE_DIM], f32, kind="ExternalInput")
    # labels pre-transposed on host to [P, N_TILES]: lab[p, t] = labels[t*P + p]
    lab_d = nc.dram_tensor("labels", [P, N_TILES], i32, kind="ExternalInput")
    cen_d = nc.dram_tensor(
        "centers", [NUM_CLASSES, FEATURE_DIM], f32, kind="ExternalInput"
    )
    out_d = nc.dram_tensor("out", [1, 1], f32, kind="ExternalOutput")

    with tile.TileContext(nc) as tc:
        with (
            tc.tile_pool(name="data", bufs=N_GROUPS) as data,
            tc.tile_pool(name="gbuf", bufs=16) as gbuf,
            tc.tile_pool(name="work", bufs=8) as work,
            tc.tile_pool(name="single", bufs=1) as single,
            tc.tile_pool(name="psum", bufs=1, space="PSUM") as psum,
        ):
            lab_all = single.tile([P, N_TILES], i32)
            nc.sync.dma_start(out=lab_all[:], in_=lab_d[:, :])

            # x group-DMAs staggered between gathers (group g+1 issued just
            # before gather 8g) so the SWDGE ring's SDMA consumption isn't
            # starved by a 4MB x flood at kernel start
            x_tiles = [None] * N_GROUPS

            def load_x_group(g):
                x_t = data.tile([P, GROUP, FEATURE_DIM], f32, tag="x")
                nc.sync.dma_start(
                    out=x_t[:],
                    in_=x_d[g * GROUP * P : (g + 1) * GROUP * P, :].rearrange(
                        "(t p) e -> p t e", p=P
                    ),
                )
                x_tiles[g] = x_t

            load_x_group(0)

            acc = single.tile([P, N_TILES], f32)
            for t in range(N_TILES):
                g, j = divmod(t, GROUP)
                if j == 0 and g + 1 < N_GROUPS and x_tiles[g + 1] is None:
                    load_x_group(g + 1)
                g_t = gbuf.tile([P, FEATURE_DIM], f32, tag="g")
                nc.gpsimd.indirect_dma_start(
                    out=g_t[:],
                    out_offset=None,
                    in_=cen_d[:, :],
                    in_offset=bass.IndirectOffsetOnAxis(
                        ap=lab_all[:, t : t + 1], axis=0
                    ),
                )
                d_t = work.tile([P, FEATURE_DIM], f32, tag="d")
                nc.vector.tensor_tensor(
                    out=d_t[:],
                    in0=x_tiles[g][:, j, :],
                    in1=g_t[:],
                    op=mybir.AluOpType.subtract,
                )
                s_t = work.tile([P, FEATURE_DIM], f32, tag="s")
                nc.scalar.activation(
                    out=s_t[:],
                    in_=d_t[:],
                    func=mybir.ActivationFunctionType.Square,
                    accum_out=acc[:, t : t + 1],
                )

            clipped = single.tile([P, N_TILES], f32)
            nc.vector.tensor_scalar(
                out=clipped[:],
                in0=acc[:],
                scalar1=float(CLAMP_MIN),
                scalar2=float(CLAMP_MAX),
                op0=mybir.AluOpType.max,
                op1=mybir.AluOpType.min,
            )
            rowsum = single.tile([P, 1], f32)
            nc.vector.reduce_sum(out=rowsum[:], in_=clipped[:], axis=mybir.AxisListType.X)

            ones = single.tile([P, 1], f32)
            nc.vector.memset(ones[:], 1.0)
            tot = psum.tile([1, 1], f32, space="PSUM")
            nc.tensor.matmul(out=tot[:], lhsT=rowsum[:], rhs=ones[:], start=True, stop=True)
            res = single.tile([1, 1], f32)
            nc.vector.tensor_copy(out=res[:], in_=tot[:])
            nc.sync.dma_start(out=out_d[:, :], in_=res[:])

    nc.finalize()
    return nc


def kernel(x: np.ndarray, centers: np.ndarray, labels: np.ndarray) -> np.ndarray:
    from concourse import bass_utils

    if "nc" not in _CACHE:
        _CACHE["nc"] = _build_nc()
    nc = _CACHE["nc"]

    x = np.ascontiguousarray(np.asarray(x, dtype=np.float32))
    centers = np.ascontiguousarray(np.asarray(centers, dtype=np.float32))
    lab = np.asarray(labels).astype(np.int64).reshape(N_CORES, N_TILES, P)

    xs = x.reshape(N_CORES, SHARD, FEATURE_DIM)
    in_maps = [
        {
            "x": np.ascontiguousarray(xs[c]),
            "labels": np.ascontiguousarray(lab[c].transpose(1, 0).astype(np.int32)),
            "centers": centers,
        }
        for c in range(N_CORES)
    ]

    rr = bass_utils.run_bass_kernel_spmd(nc, in_maps, list(range(N_CORES)))
    _CACHE["last_results"] = rr

    total = sum(float(r["out"][0, 0]) for r in rr.results)
    loss = (total + BATCH * (NUM_CLASSES - 1) * CLAMP_MIN) / BATCH
    return np.asarray(loss, dtype=np.float32)
